# revision 33
# baseline (speedup 1.0000x reference)
"""Trainium2 Bass kernel for nn_DHGNNLayer (gnn_message_passing).

Math (from the reference):
    h   = relu(B1 @ x @ W1)            # [n_nodes, 128], B1 = COO incidence
    out = mean_e sigmoid((hw0[r_{2e}] + hw0[r_{2e+1}]) / 2)   # scalar
    where hw0 = relu(h) @ W2[:, 0]     # only column 0 is ever needed

Key facts used:
  - inc_cols == arange(NNZ)//2  -> every edge has exactly 2 nonzeros, deg == 2,
    and the two nonzeros of edge e are adjacent (2e, 2e+1) in original order.

Strategy (8 cores, 1D node-partition parallelism, no collectives):
  Launch A: host sorts nonzeros by destination node and gathers x rows per
    nonzero (bf16).  Nodes are split into 128-wide blocks; blocks are sorted
    by nnz count and dealt round-robin to (core, slot) so every core runs an
    identical program (SPMD) with per-slot tile counts R_j.  Per 128-nnz
    tile, a one-hot G[k, j] = (off[k] == j) is either built on the DVE
    (iota + tensor_scalar is_equal) or shipped from the host as fp8 (exact
    0/1) — a tunable DMA-vs-DVE tradeoff.  The tensor engine accumulates
    hxT_block += xg_tile^T @ G in PSUM.  Then hT = W1^T @ hxT (stationary
    W1), ReLU, and hw0_block = reluT_block^T @ W2[:,0].
  Launch B: host gathers hw0[inc_rows] (1.6 MB), device does
    sigmoid(0.5*(a+b)) and reduces; host combines 8 partial sums.
"""

import numpy as np
import ml_dtypes

N_NODES = 50000
N_EDGES = 200000
C = 128
NNZ = 2 * N_EDGES
NCORES = 8
BLK = 128                      # nodes per block (PSUM window)
NBLK = 392                     # ceil(50000/128) padded to a multiple of 8
NSLOT = NBLK // NCORES         # 49 node blocks (slots) per core
NODES_PAD = NBLK * BLK         # 50176
GRP = 64                       # xg tiles per DMA group (1 MiB fp8)
GRP8 = 64                      # gq tiles per DMA group (1 MiB fp8)
SHIP_NUM = 3                   # ship G for tiles with t % SHIP_DEN < SHIP_NUM
SHIP_DEN = 5
XG_FP8 = True                  # ship xg as fp8e4m3 (halves the big DMA)
WSTRIP = 4                     # slots per w1/relu strip
FP8_ONE = np.uint8(0x38)       # float8_e4m3 encoding of 1.0

_PROGS = {}
TRACE = False
LAST = {}


def _shipped(t):
    return (t % SHIP_DEN) < SHIP_NUM


def _bacc():
    import concourse.bacc as bacc

    return bacc.Bacc("TRN2", target_bir_lowering=False, debug=False,
                     num_devices=NCORES)


def _build_prog_a(rj, ntp, nship, ntiles, nstiles):
    """Layer-1 program: segment-sum + W1 + relu + W2[:,0] per node block."""
    import concourse.mybir as mybir
    from concourse import tile

    dtb = mybir.dt.bfloat16
    dtf = mybir.dt.float32
    dt8 = mybir.dt.float8e4
    dtx = dt8 if XG_FP8 else dtb
    AF = mybir.ActivationFunctionType
    NFREE = NSLOT * BLK        # 6272 nodes per core

    nc = _bacc()
    xg_d = nc.dram_tensor("xg", [128, ntp, C], dtx, kind="ExternalInput")
    off_d = nc.dram_tensor("off", [128, ntp], dtf, kind="ExternalInput")
    gq_d = nc.dram_tensor("gq", [128, nship, 128], dt8, kind="ExternalInput")
    w1_d = nc.dram_tensor("w1", [C, C], dtb, kind="ExternalInput")
    w2c_d = nc.dram_tensor("w2c", [C, 1], dtb, kind="ExternalInput")
    hw0_d = nc.dram_tensor("hw0", [1, NFREE], dtf, kind="ExternalOutput")

    with tile.TileContext(nc) as tc:
        with (
            tc.tile_pool(name="const", bufs=1) as constp,
            tc.tile_pool(name="xgp", bufs=4) as xgp,
            tc.tile_pool(name="gqp", bufs=3) as gqp,
            tc.tile_pool(name="gp", bufs=12) as gp,
            tc.tile_pool(name="rlp", bufs=4) as rlp,
            tc.tile_pool(name="hwp", bufs=3) as hwp,
            tc.tile_pool(name="ps_hx", bufs=4, space="PSUM") as ps_hx,
            tc.tile_pool(name="ps_h", bufs=2, space="PSUM") as ps_h,
            tc.tile_pool(name="ps_o", bufs=2, space="PSUM") as ps_o,
        ):
            iota_t = constp.tile([128, 128], dtb)
            nc.gpsimd.iota(iota_t[:], [[1, 128]], channel_multiplier=0,
                           allow_small_or_imprecise_dtypes=True)
            # off table in independent chunks so the first is_eq only
            # waits for a 32KB DMA, not the whole table
            OFFC = 64
            off_tiles = []
            for o0 in range(0, ntp, OFFC):
                w = min(OFFC, ntp - o0)
                ot = constp.tile([128, OFFC], dtf, tag=f"off{o0}")
                nc.sync.dma_start(ot[:, :w], off_d[:, o0:o0 + w])
                off_tiles.append(ot)
            w1_sb = constp.tile([C, C], dtb)
            nc.sync.dma_start(w1_sb[:], w1_d[:])
            w2c_sb = constp.tile([C, 1], dtb)
            nc.sync.dma_start(w2c_sb[:], w2c_d[:])

            hxT_sb = constp.tile([128, NFREE], dtb)

            def w1_strip(s0, w):
                # hT strip = W1^T @ hxT[:, s0:s0+w], relu, then
                # hw0 strip = w2col^T @ reluT (M=1 stationary, cheap ld)
                psh = ps_h.tile([C, 512], dtf, tag="h")
                nc.tensor.matmul(psh[:, :w], w1_sb[:], hxT_sb[:, s0:s0 + w],
                                 start=True, stop=True)
                reluT_sb = rlp.tile([128, 512], dtb, tag="reluT")
                nc.scalar.activation(reluT_sb[:, :w], psh[:, :w], AF.Relu)
                pso = ps_o.tile([1, 512], dtf, tag="o")
                nc.tensor.matmul(pso[:, :w], w2c_sb[:], reluT_sb[:, :w],
                                 start=True, stop=True)
                hw0_sb = hwp.tile([1, 512], dtf, tag="hw0")
                nc.scalar.activation(hw0_sb[:, :w], pso[:, :w], AF.Copy)
                # scalar-issued DMA rides qActDynamicHW — keeps the output
                # out of the sync engine's prefetch FIFO
                nc.scalar.dma_start(hw0_d[:, s0:s0 + w], hw0_sb[:, :w])

            # staggered group schedule: small first groups so the first
            # matmul starts after a ~100KB DMA, not a 1MB one
            def bounds(total, first, full):
                # [first, full-first, full, full, ...] keeps later groups
                # aligned to `full` boundaries of the dram layout
                out = [(0, min(first, total))]
                b = out[0][1]
                if b < total and b % full:
                    n = min(full - b % full, total - b)
                    out.append((b, n))
                    b += n
                while b < total:
                    n = min(full, total - b)
                    out.append((b, n))
                    b += n
                return out

            xg_bounds = bounds(ntiles, 8, GRP)
            gq_bounds = bounds(nstiles, 4, GRP8)
            xg_next = 0
            gq_next = 0
            xg_map = {}
            gq_map = {}
            cur_xt = None
            cur_gq = None
            t = 0
            s = 0
            for j in range(NSLOT):
                r = rj[j]
                psum_hx = ps_hx.tile([C, BLK], dtf, tag="hx")
                for i in range(r):
                    if xg_next < len(xg_bounds) and t == xg_bounds[xg_next][0]:
                        b0, n = xg_bounds[xg_next]
                        cur_xt = xgp.tile([128, GRP, C], dtx, tag="xg")
                        nc.sync.dma_start(cur_xt[:, :n, :],
                                          xg_d[:, b0:b0 + n, :])
                        xg_map = {b0 + q: q for q in range(n)}
                        xg_next += 1
                    if _shipped(t):
                        if gq_next < len(gq_bounds) and \
                                s == gq_bounds[gq_next][0]:
                            b0, n = gq_bounds[gq_next]
                            cur_gq = gqp.tile([128, GRP8, 128], dt8, tag="gq")
                            nc.sync.dma_start(cur_gq[:, :n, :],
                                              gq_d[:, b0:b0 + n, :])
                            gq_map = {b0 + q: q for q in range(n)}
                            gq_next += 1
                        g_ap = cur_gq[:, gq_map[s], :]
                        s += 1
                    else:
                        g_sb = gp.tile([128, 128], dtb, tag="G")
                        nc.vector.tensor_scalar(
                            g_sb[:], iota_t[:],
                            off_tiles[t // OFFC][:, t % OFFC:t % OFFC + 1],
                            None, mybir.AluOpType.is_equal)
                        g_ap = g_sb[:]
                    # psum_hx[c, j] += sum_k xg[k, c] * G[k, j]
                    nc.tensor.matmul(psum_hx[:], cur_xt[:, xg_map[t], :],
                                     g_ap, start=(i == 0), stop=(i == r - 1))
                    t += 1
                nc.scalar.activation(hxT_sb[:, j * BLK:(j + 1) * BLK],
                                     psum_hx[:], AF.Copy)
                if (j + 1) % WSTRIP == 0:
                    w1_strip((j + 1 - WSTRIP) * BLK, WSTRIP * BLK)
            rem = NSLOT % WSTRIP
            if rem:
                w1_strip((NSLOT - rem) * BLK, rem * BLK)

    nc.compile()
    return nc


def _build_prog_b(free):
    """Layer-2 program (raw bass, minimal tail):
    acc[p] = sum_f sigmoid(0.5*(a+b)).  zab is [za | zb] along free."""
    import concourse.bass as bass
    import concourse.mybir as mybir

    dtb = mybir.dt.bfloat16
    dtf = mybir.dt.float32
    AF = mybir.ActivationFunctionType

    nc = bass.Bass()
    zab_d = nc.dram_tensor("zab", [128, 2 * free], dtb, kind="ExternalInput")
    acc_d = nc.dram_tensor("acc", [128, 1], dtf, kind="ExternalOutput")

    with (
        nc.sbuf_tensor([128, 2 * free], dtb) as zab_sb,
        nc.sbuf_tensor([128, free], dtf) as t_sb,
        nc.sbuf_tensor([128, free], dtf) as s_sb,
        nc.sbuf_tensor([128, 1], dtf) as r_sb,
        nc.semaphore() as dsem,
        nc.semaphore() as csem,
        nc.Block() as block,
    ):
        @block.sync
        def _(sync):
            sync.dma_start(zab_sb[:], zab_d[:]).then_inc(dsem, 16)
            sync.wait_ge(csem, 2)
            sync.dma_start(acc_d[:], r_sb[:]).then_inc(dsem, 16)

        @block.vector
        def _(vector):
            vector.wait_ge(dsem, 16)
            nc.vector.tensor_add(t_sb[:], zab_sb[:, :free],
                                 zab_sb[:, free:]).then_inc(csem, 1)

        @block.scalar
        def _(scalar):
            scalar.wait_ge(csem, 1)
            nc.scalar.activation(s_sb[:], t_sb[:], AF.Sigmoid, scale=0.5,
                                 accum_out=r_sb[:]).then_inc(csem, 1)

    return nc


def _get_prog(key, builder, *args):
    if key not in _PROGS:
        _PROGS[key] = builder(*args)
    return _PROGS[key]


def _run(nc, in_maps, tag):
    from concourse.bass_utils import run_bass_kernel_spmd
    import time

    t0 = time.perf_counter()
    res = run_bass_kernel_spmd(nc, in_maps, list(range(NCORES)), trace=TRACE)
    LAST[tag + "_wall_s"] = time.perf_counter() - t0
    LAST[tag + "_exec_ns"] = res.exec_time_ns
    return res.results


def kernel(x, w1, w2, inc_rows, inc_cols, n_nodes=None, n_edges=None):
    x = np.asarray(x, dtype=np.float32)
    w1 = np.asarray(w1, dtype=np.float32)
    w2 = np.asarray(w2, dtype=np.float32)
    inc_rows = np.asarray(inc_rows)
    inc_cols = np.asarray(inc_cols)
    assert x.shape == (N_EDGES, C) and inc_rows.shape == (NNZ,)
    # every edge contributes exactly its two adjacent nonzeros (deg == 2)
    assert np.array_equal(inc_cols.astype(np.int64),
                          np.arange(NNZ, dtype=np.int64) // 2)

    # ---- host prep for launch A: sort nnz by destination node ----
    order = np.argsort(inc_rows, kind="stable")
    rs = inc_rows[order].astype(np.int64)
    cs = inc_cols[order].astype(np.int64)

    blk = rs >> 7
    counts = np.bincount(blk, minlength=NBLK)
    starts = np.zeros(NBLK, np.int64)
    starts[1:] = np.cumsum(counts)[:-1]

    # sorted block -> (core, slot) assignment; per-slot tile count rj
    ordb = np.argsort(-counts, kind="stable")          # NBLK block ids
    pos = np.empty(NBLK, np.int64)
    pos[ordb] = np.arange(NBLK)                        # block -> rank
    slot_of_blk = pos // NCORES
    core_of_blk = pos % NCORES
    slot_counts = counts[ordb].reshape(NSLOT, NCORES)
    rj = np.maximum(1, -(-slot_counts.max(axis=1) // 128)).astype(int)
    toff = np.zeros(NSLOT, np.int64)
    toff[1:] = np.cumsum(rj)[:-1]
    T = int(rj.sum())
    NTP = -(-T // GRP) * GRP
    ship_mask_t = np.array([_shipped(t) for t in range(T)], dtype=bool)
    ship_idx_t = np.cumsum(ship_mask_t) - 1            # tile -> gq slot
    NSH = int(ship_mask_t.sum())
    NSHIP = -(-NSH // GRP8) * GRP8

    # per-nnz destination coordinates
    k = np.arange(NNZ, dtype=np.int64)
    w_in_blk = k - starts[blk]
    core_k = core_of_blk[blk]
    slot_k = slot_of_blk[blk]
    tile_k = toff[slot_k] + (w_in_blk >> 7)            # per-core tile index
    p_k = w_in_blk & 127
    o_k = (rs & 127)

    xdt = ml_dtypes.float8_e4m3 if XG_FP8 else ml_dtypes.bfloat16
    xbf = x.astype(xdt)
    xg_cores = np.zeros((NCORES, 128, NTP, C), dtype=xdt)
    xg_cores[core_k, p_k, tile_k, :] = xbf[cs]
    off_cores = np.zeros((NCORES, 128, NTP), dtype=np.float32)
    off_cores[core_k, p_k, tile_k] = o_k.astype(np.float32)

    gq_cores = np.zeros((NCORES, 128, NSHIP, 128), dtype=np.uint8)
    shipped_k = ship_mask_t[tile_k]
    gq_cores[core_k[shipped_k], p_k[shipped_k],
             ship_idx_t[tile_k[shipped_k]], o_k[shipped_k]] = FP8_ONE
    gq_cores = gq_cores.view(ml_dtypes.float8_e4m3)

    w1b = w1.astype(ml_dtypes.bfloat16)
    w2cb = w2[:, 0:1].astype(ml_dtypes.bfloat16)

    prog_a = _get_prog(("A", tuple(rj), NTP, NSHIP), _build_prog_a,
                       rj, NTP, NSHIP, T, NSH)
    in_maps = [{"xg": xg_cores[m], "off": off_cores[m], "gq": gq_cores[m],
                "w1": w1b, "w2c": w2cb} for m in range(NCORES)]
    res_a = _run(prog_a, in_maps, "A")

    # ---- host glue: assemble hw0, gather per-nonzero values ----
    # per-core hw0 row: [1, 6272], local node = 128*slot + p
    parts = np.stack([res_a[m]["hw0"].reshape(NSLOT, 128)
                      for m in range(NCORES)])                  # [8,49,128]
    by_rank = parts.transpose(1, 0, 2).reshape(NBLK, 128)       # rank-major
    hw0 = np.empty((NBLK, 128), dtype=np.float32)
    hw0[ordb] = by_rank
    hw0 = hw0.reshape(-1)
    zg = hw0[inc_rows.astype(np.int64)]
    za = zg[0::2]
    zb = zg[1::2]

    # ---- launch B: sigmoid + reduce ----
    FREE = -(-N_EDGES // (NCORES * 128))               # 196
    tot = NCORES * 128 * FREE
    zap = np.full(tot, -1.0e4, np.float32)
    zbp = np.full(tot, -1.0e4, np.float32)
    zap[:N_EDGES] = za
    zbp[:N_EDGES] = zb
    zab = np.concatenate(
        [zap.reshape(NCORES, 128, FREE), zbp.reshape(NCORES, 128, FREE)],
        axis=2).astype(ml_dtypes.bfloat16)

    prog_b = _get_prog(("B", FREE), _build_prog_b, FREE)
    in_maps_b = [{"zab": zab[m]} for m in range(NCORES)]
    res_b = _run(prog_b, in_maps_b, "B")

    total = float(sum(float(r["acc"].sum()) for r in res_b))
    return np.array(total / N_EDGES, dtype=np.float32)


# revision 41
# speedup vs baseline: 1.1244x; 1.1244x over previous
"""Trainium2 Bass kernel for nn_DHGNNLayer (gnn_message_passing).

Math (from the reference):
    h   = relu(B1 @ x @ W1)            # [n_nodes, 128], B1 = COO incidence
    out = mean_e sigmoid((hw0[r_{2e}] + hw0[r_{2e+1}]) / 2)   # scalar
    where hw0 = relu(h) @ W2[:, 0]     # only column 0 is ever needed

Key facts used:
  - inc_cols == arange(NNZ)//2  -> every edge has exactly 2 nonzeros, deg == 2,
    and the two nonzeros of edge e are adjacent (2e, 2e+1) in original order.

Strategy (8 cores, 1D node-partition parallelism, no collectives):
  Launch A: host sorts nonzeros by destination node and gathers x rows per
    nonzero (bf16).  Nodes are split into 128-wide blocks; blocks are sorted
    by nnz count and dealt round-robin to (core, slot) so every core runs an
    identical program (SPMD) with per-slot tile counts R_j.  Per 128-nnz
    tile, a one-hot G[k, j] = (off[k] == j) is either built on the DVE
    (iota + tensor_scalar is_equal) or shipped from the host as fp8 (exact
    0/1) — a tunable DMA-vs-DVE tradeoff.  The tensor engine accumulates
    hxT_block += xg_tile^T @ G in PSUM.  Then hT = W1^T @ hxT (stationary
    W1), ReLU, and hw0_block = reluT_block^T @ W2[:,0].
  Launch B: host gathers hw0[inc_rows] (1.6 MB), device does
    sigmoid(0.5*(a+b)) and reduces; host combines 8 partial sums.
"""

import numpy as np
import ml_dtypes

N_NODES = 50000
N_EDGES = 200000
C = 128
NNZ = 2 * N_EDGES
NCORES = 8
BLK = 128                      # nodes per block (PSUM window)
NBLK = 392                     # ceil(50000/128) padded to a multiple of 8
NSLOT = NBLK // NCORES         # 49 node blocks (slots) per core
NODES_PAD = NBLK * BLK         # 50176
GRP = 64                       # xg tiles per DMA group (1 MiB fp8)
GRP8 = 64                      # gq tiles per DMA group (1 MiB fp8)
SHIP_NUM = 1                   # ship G for tiles with t % SHIP_DEN < SHIP_NUM
SHIP_DEN = 2
XG_FP8 = True                  # ship xg as fp8e4m3 (halves the big DMA)
WSTRIP = 4                     # slots per w1/relu strip
FP8_ONE = np.uint8(0x38)       # float8_e4m3 encoding of 1.0

_PROGS = {}
TRACE = False
LAST = {}


def _shipped(t):
    return (t % SHIP_DEN) < SHIP_NUM


def _bacc():
    import concourse.bacc as bacc

    return bacc.Bacc("TRN2", target_bir_lowering=False, debug=False,
                     num_devices=NCORES)


def _build_prog_a(rj, ntp, nship, ntiles, nstiles):
    """Layer-1 program: segment-sum + W1 + relu + W2[:,0] per node block."""
    import concourse.mybir as mybir
    from concourse import tile

    dtb = mybir.dt.bfloat16
    dtf = mybir.dt.float32
    dt8 = mybir.dt.float8e4
    dtx = dt8 if XG_FP8 else dtb
    AF = mybir.ActivationFunctionType
    NFREE = NSLOT * BLK        # 6272 nodes per core

    nc = _bacc()
    xg_d = nc.dram_tensor("xg", [128, ntp, C], dtx, kind="ExternalInput")
    off_d = nc.dram_tensor("off", [128, ntp], dtf, kind="ExternalInput")
    gq_d = nc.dram_tensor("gq", [128, nship, 128], dt8, kind="ExternalInput")
    w1_d = nc.dram_tensor("w1", [C, C], dtb, kind="ExternalInput")
    w2c_d = nc.dram_tensor("w2c", [C, 1], dtb, kind="ExternalInput")
    hw0_d = nc.dram_tensor("hw0", [1, NFREE], dtf, kind="ExternalOutput")

    with tile.TileContext(nc) as tc:
        with (
            tc.tile_pool(name="const", bufs=1) as constp,
            tc.tile_pool(name="xgp", bufs=4) as xgp,
            tc.tile_pool(name="gqp", bufs=3) as gqp,
            tc.tile_pool(name="gp", bufs=12) as gp,
            tc.tile_pool(name="rlp", bufs=4) as rlp,
            tc.tile_pool(name="ps_hx", bufs=4, space="PSUM") as ps_hx,
            tc.tile_pool(name="ps_h", bufs=2, space="PSUM") as ps_h,
            tc.tile_pool(name="ps_o", bufs=2, space="PSUM") as ps_o,
        ):
            iota_t = constp.tile([128, 128], dtb)
            nc.gpsimd.iota(iota_t[:], [[1, 128]], channel_multiplier=0,
                           allow_small_or_imprecise_dtypes=True)
            off_sb = constp.tile([128, ntp], dtf)
            nc.sync.dma_start(off_sb[:], off_d[:])
            w1_sb = constp.tile([C, C], dtb)
            nc.sync.dma_start(w1_sb[:], w1_d[:])
            w2c_sb = constp.tile([C, 1], dtb)
            nc.sync.dma_start(w2c_sb[:], w2c_d[:])

            hxT_sb = constp.tile([128, NFREE], dtb)
            hw0_sb = constp.tile([1, NFREE], dtf)

            def w1_strip(s0, w):
                # hT strip = W1^T @ hxT[:, s0:s0+w], relu, then
                # hw0 strip = w2col^T @ reluT (M=1 stationary, cheap ld)
                psh = ps_h.tile([C, 512], dtf, tag="h")
                nc.tensor.matmul(psh[:, :w], w1_sb[:], hxT_sb[:, s0:s0 + w],
                                 start=True, stop=True)
                reluT_sb = rlp.tile([128, 512], dtb, tag="reluT")
                nc.scalar.activation(reluT_sb[:, :w], psh[:, :w], AF.Relu)
                pso = ps_o.tile([1, 512], dtf, tag="o")
                nc.tensor.matmul(pso[:, :w], w2c_sb[:], reluT_sb[:, :w],
                                 start=True, stop=True)
                nc.scalar.activation(hw0_sb[:, s0:s0 + w], pso[:, :w],
                                     AF.Copy)

            def bounds(total, full):
                out = []
                b = 0
                while b < total:
                    n = min(full, total - b)
                    out.append((b, n))
                    b += n
                return out

            xg_bounds = bounds(ntiles, GRP)
            gq_bounds = bounds(nstiles, GRP8)
            xg_next = 0
            gq_next = 0
            xg_map = {}
            gq_map = {}
            cur_xt = None
            cur_gq = None
            t = 0
            s = 0
            for j in range(NSLOT):
                r = rj[j]
                psum_hx = ps_hx.tile([C, BLK], dtf, tag="hx")
                for i in range(r):
                    if xg_next < len(xg_bounds) and t == xg_bounds[xg_next][0]:
                        b0, n = xg_bounds[xg_next]
                        cur_xt = xgp.tile([128, GRP, C], dtx, tag="xg")
                        nc.sync.dma_start(cur_xt[:, :n, :],
                                          xg_d[:, b0:b0 + n, :])
                        xg_map = {b0 + q: q for q in range(n)}
                        xg_next += 1
                    if _shipped(t):
                        if gq_next < len(gq_bounds) and \
                                s == gq_bounds[gq_next][0]:
                            b0, n = gq_bounds[gq_next]
                            cur_gq = gqp.tile([128, GRP8, 128], dt8, tag="gq")
                            nc.sync.dma_start(cur_gq[:, :n, :],
                                              gq_d[:, b0:b0 + n, :])
                            gq_map = {b0 + q: q for q in range(n)}
                            gq_next += 1
                        g_ap = cur_gq[:, gq_map[s], :]
                        s += 1
                    else:
                        g_sb = gp.tile([128, 128], dtb, tag="G")
                        nc.vector.tensor_scalar(
                            g_sb[:], iota_t[:], off_sb[:, t:t + 1], None,
                            mybir.AluOpType.is_equal)
                        g_ap = g_sb[:]
                    # psum_hx[c, j] += sum_k xg[k, c] * G[k, j]
                    nc.tensor.matmul(psum_hx[:], cur_xt[:, xg_map[t], :],
                                     g_ap, start=(i == 0), stop=(i == r - 1))
                    t += 1
                nc.scalar.activation(hxT_sb[:, j * BLK:(j + 1) * BLK],
                                     psum_hx[:], AF.Copy)
                if (j + 1) % WSTRIP == 0:
                    w1_strip((j + 1 - WSTRIP) * BLK, WSTRIP * BLK)
            rem = NSLOT % WSTRIP
            if rem:
                w1_strip((NSLOT - rem) * BLK, rem * BLK)

            nc.sync.dma_start(hw0_d[:], hw0_sb[:])

    nc.compile()
    return nc


def _build_prog_b(free):
    """Layer-2 program (raw bass, minimal tail):
    acc[p] = sum_f sigmoid(0.5*(a+b)).  zab is [za | zb] along free."""
    import concourse.bass as bass
    import concourse.mybir as mybir

    dtb = mybir.dt.bfloat16
    dtf = mybir.dt.float32
    AF = mybir.ActivationFunctionType

    nc = bass.Bass()
    zab_d = nc.dram_tensor("zab", [128, 2 * free], dtb, kind="ExternalInput")
    acc_d = nc.dram_tensor("acc", [128, 1], dtf, kind="ExternalOutput")

    with (
        nc.sbuf_tensor([128, 2 * free], dtb) as zab_sb,
        nc.sbuf_tensor([128, free], dtf) as t_sb,
        nc.sbuf_tensor([128, free], dtf) as s_sb,
        nc.sbuf_tensor([128, 1], dtf) as r_sb,
        nc.semaphore() as dsem,
        nc.semaphore() as csem,
        nc.Block() as block,
    ):
        @block.sync
        def _(sync):
            sync.dma_start(zab_sb[:], zab_d[:]).then_inc(dsem, 16)
            sync.wait_ge(csem, 2)
            sync.dma_start(acc_d[:], r_sb[:]).then_inc(dsem, 16)

        @block.vector
        def _(vector):
            vector.wait_ge(dsem, 16)
            nc.vector.tensor_add(t_sb[:], zab_sb[:, :free],
                                 zab_sb[:, free:]).then_inc(csem, 1)

        @block.scalar
        def _(scalar):
            scalar.wait_ge(csem, 1)
            nc.scalar.activation(s_sb[:], t_sb[:], AF.Sigmoid, scale=0.5,
                                 accum_out=r_sb[:]).then_inc(csem, 1)

    return nc


def _get_prog(key, builder, *args):
    if key not in _PROGS:
        _PROGS[key] = builder(*args)
    return _PROGS[key]


def _run(nc, in_maps, tag):
    from concourse.bass_utils import run_bass_kernel_spmd
    import time

    t0 = time.perf_counter()
    res = run_bass_kernel_spmd(nc, in_maps, list(range(NCORES)), trace=TRACE)
    LAST[tag + "_wall_s"] = time.perf_counter() - t0
    LAST[tag + "_exec_ns"] = res.exec_time_ns
    return res.results


def kernel(x, w1, w2, inc_rows, inc_cols, n_nodes=None, n_edges=None):
    x = np.asarray(x, dtype=np.float32)
    w1 = np.asarray(w1, dtype=np.float32)
    w2 = np.asarray(w2, dtype=np.float32)
    inc_rows = np.asarray(inc_rows)
    inc_cols = np.asarray(inc_cols)
    assert x.shape == (N_EDGES, C) and inc_rows.shape == (NNZ,)
    # every edge contributes exactly its two adjacent nonzeros (deg == 2)
    assert np.array_equal(inc_cols.astype(np.int64),
                          np.arange(NNZ, dtype=np.int64) // 2)

    # ---- host prep for launch A: sort nnz by destination node ----
    order = np.argsort(inc_rows, kind="stable")
    rs = inc_rows[order].astype(np.int64)
    cs = inc_cols[order].astype(np.int64)

    blk = rs >> 7
    counts = np.bincount(blk, minlength=NBLK)
    starts = np.zeros(NBLK, np.int64)
    starts[1:] = np.cumsum(counts)[:-1]

    # sorted block -> (core, slot) assignment; per-slot tile count rj
    ordb = np.argsort(-counts, kind="stable")          # NBLK block ids
    pos = np.empty(NBLK, np.int64)
    pos[ordb] = np.arange(NBLK)                        # block -> rank
    slot_of_blk = pos // NCORES
    core_of_blk = pos % NCORES
    slot_counts = counts[ordb].reshape(NSLOT, NCORES)
    rj = np.maximum(1, -(-slot_counts.max(axis=1) // 128)).astype(int)
    toff = np.zeros(NSLOT, np.int64)
    toff[1:] = np.cumsum(rj)[:-1]
    T = int(rj.sum())
    NTP = -(-T // GRP) * GRP
    ship_mask_t = np.array([_shipped(t) for t in range(T)], dtype=bool)
    ship_idx_t = np.cumsum(ship_mask_t) - 1            # tile -> gq slot
    NSH = int(ship_mask_t.sum())
    NSHIP = -(-NSH // GRP8) * GRP8

    # per-nnz destination coordinates
    k = np.arange(NNZ, dtype=np.int64)
    w_in_blk = k - starts[blk]
    core_k = core_of_blk[blk]
    slot_k = slot_of_blk[blk]
    tile_k = toff[slot_k] + (w_in_blk >> 7)            # per-core tile index
    p_k = w_in_blk & 127
    o_k = (rs & 127)

    xdt = ml_dtypes.float8_e4m3 if XG_FP8 else ml_dtypes.bfloat16
    xbf = x.astype(xdt)
    xg_cores = np.zeros((NCORES, 128, NTP, C), dtype=xdt)
    xg_cores[core_k, p_k, tile_k, :] = xbf[cs]
    off_cores = np.zeros((NCORES, 128, NTP), dtype=np.float32)
    off_cores[core_k, p_k, tile_k] = o_k.astype(np.float32)

    gq_cores = np.zeros((NCORES, 128, NSHIP, 128), dtype=np.uint8)
    shipped_k = ship_mask_t[tile_k]
    gq_cores[core_k[shipped_k], p_k[shipped_k],
             ship_idx_t[tile_k[shipped_k]], o_k[shipped_k]] = FP8_ONE
    gq_cores = gq_cores.view(ml_dtypes.float8_e4m3)

    w1b = w1.astype(ml_dtypes.bfloat16)
    w2cb = w2[:, 0:1].astype(ml_dtypes.bfloat16)

    prog_a = _get_prog(("A", tuple(rj), NTP, NSHIP), _build_prog_a,
                       rj, NTP, NSHIP, T, NSH)
    in_maps = [{"xg": xg_cores[m], "off": off_cores[m], "gq": gq_cores[m],
                "w1": w1b, "w2c": w2cb} for m in range(NCORES)]
    res_a = _run(prog_a, in_maps, "A")

    # ---- host glue: assemble hw0, gather per-nonzero values ----
    # per-core hw0 row: [1, 6272], local node = 128*slot + p
    parts = np.stack([res_a[m]["hw0"].reshape(NSLOT, 128)
                      for m in range(NCORES)])                  # [8,49,128]
    by_rank = parts.transpose(1, 0, 2).reshape(NBLK, 128)       # rank-major
    hw0 = np.empty((NBLK, 128), dtype=np.float32)
    hw0[ordb] = by_rank
    hw0 = hw0.reshape(-1)
    zg = hw0[inc_rows.astype(np.int64)]
    za = zg[0::2]
    zb = zg[1::2]

    # ---- launch B: sigmoid + reduce ----
    FREE = -(-N_EDGES // (NCORES * 128))               # 196
    tot = NCORES * 128 * FREE
    zap = np.full(tot, -1.0e4, np.float32)
    zbp = np.full(tot, -1.0e4, np.float32)
    zap[:N_EDGES] = za
    zbp[:N_EDGES] = zb
    zab = np.concatenate(
        [zap.reshape(NCORES, 128, FREE), zbp.reshape(NCORES, 128, FREE)],
        axis=2).astype(ml_dtypes.bfloat16)

    prog_b = _get_prog(("B", FREE), _build_prog_b, FREE)
    in_maps_b = [{"zab": zab[m]} for m in range(NCORES)]
    res_b = _run(prog_b, in_maps_b, "B")

    total = float(sum(float(r["acc"].sum()) for r in res_b))
    return np.array(total / N_EDGES, dtype=np.float32)


# revision 45
# speedup vs baseline: 1.1743x; 1.0444x over previous
"""Trainium2 Bass kernel for nn_DHGNNLayer (gnn_message_passing).

Math (from the reference):
    h   = relu(B1 @ x @ W1)            # [n_nodes, 128], B1 = COO incidence
    out = mean_e sigmoid((hw0[r_{2e}] + hw0[r_{2e+1}]) / 2)   # scalar
    where hw0 = relu(h) @ W2[:, 0]     # only column 0 is ever needed

Key facts used:
  - inc_cols == arange(NNZ)//2  -> every edge has exactly 2 nonzeros, deg == 2,
    and the two nonzeros of edge e are adjacent (2e, 2e+1) in original order.

Strategy (8 cores, 1D node-partition parallelism, no collectives):
  Launch A: host sorts nonzeros by destination node and gathers x rows per
    nonzero (fp8e4m3).  Nodes are split into 128-wide blocks; blocks are sorted
    by nnz count and dealt round-robin to (core, slot) so every core runs an
    identical program (SPMD) with per-slot tile counts R_j.  Per 128-nnz
    tile, a one-hot G[k, j] = (off[k] == j) is either built on the DVE
    (iota + tensor_scalar is_equal) or shipped from the host as fp8 (exact
    0/1) — a tunable DMA-vs-DVE tradeoff.  The tensor engine accumulates
    hxT_block += xg_tile^T @ G in PSUM.  Then hT = W1^T @ hxT (stationary
    W1), ReLU, and hw0_block = reluT_block^T @ W2[:,0].
  Launch B: host gathers hw0[inc_rows] (1.6 MB), device does
    sigmoid(0.5*(a+b)) and reduces; host combines 8 partial sums.
"""

import numpy as np
import ml_dtypes

N_NODES = 50000
N_EDGES = 200000
C = 128
NNZ = 2 * N_EDGES
NCORES = 8
BLK = 128                      # nodes per block (PSUM window)
NBLK = 392                     # ceil(50000/128) padded to a multiple of 8
NSLOT = NBLK // NCORES         # 49 node blocks (slots) per core
NODES_PAD = NBLK * BLK         # 50176
GRP = 64                       # xg tiles per DMA group (1 MiB fp8)
GRP8 = 64                      # gq tiles per DMA group (1 MiB fp8)
SHIP_NUM = 1                   # ship G for tiles with t % SHIP_DEN < SHIP_NUM
SHIP_DEN = 2
XG_FP8 = True                  # ship xg as fp8e4m3 (halves the big DMA)
WSTRIP = 4                     # slots per w1/relu strip
FP8_ONE = np.uint8(0x38)       # float8_e4m3 encoding of 1.0

_PROGS = {}
TRACE = False
LAST = {}


def _shipped(t):
    return (t % SHIP_DEN) < SHIP_NUM


def _bacc():
    import concourse.bacc as bacc

    return bacc.Bacc("TRN2", target_bir_lowering=False, debug=False,
                     num_devices=NCORES)


def _build_prog_a(rj, ntp, nship, ntiles, nstiles):
    """Layer-1 program: segment-sum + W1 + relu + W2[:,0] per node block."""
    import concourse.mybir as mybir
    from concourse import tile

    dtb = mybir.dt.bfloat16
    dtf = mybir.dt.float32
    dt8 = mybir.dt.float8e4
    dtx = dt8 if XG_FP8 else dtb
    AF = mybir.ActivationFunctionType
    NFREE = NSLOT * BLK        # 6272 nodes per core

    nc = _bacc()
    xg_d = nc.dram_tensor("xg", [128, ntp, C], dtx, kind="ExternalInput")
    off_d = nc.dram_tensor("off", [128, ntp], dtf, kind="ExternalInput")
    gq_d = nc.dram_tensor("gq", [128, nship, 128], dt8, kind="ExternalInput")
    w1_d = nc.dram_tensor("w1", [C, C], dtb, kind="ExternalInput")
    w2c_d = nc.dram_tensor("w2c", [C, 1], dtb, kind="ExternalInput")
    hw0_d = nc.dram_tensor("hw0", [1, NFREE], dtf, kind="ExternalOutput")

    with tile.TileContext(nc) as tc:
        with (
            tc.tile_pool(name="const", bufs=1) as constp,
            tc.tile_pool(name="xgp", bufs=4) as xgp,
            tc.tile_pool(name="gqp", bufs=4) as gqp,
            tc.tile_pool(name="gp", bufs=16) as gp,
            tc.tile_pool(name="rlp", bufs=4) as rlp,
            tc.tile_pool(name="ps_hx", bufs=4, space="PSUM") as ps_hx,
            tc.tile_pool(name="ps_h", bufs=2, space="PSUM") as ps_h,
            tc.tile_pool(name="ps_o", bufs=2, space="PSUM") as ps_o,
        ):
            def bounds(total, full):
                out = []
                b = 0
                while b < total:
                    n = min(full, total - b)
                    out.append((b, n))
                    b += n
                return out

            xg_bounds = bounds(ntiles, GRP)
            gq_bounds = bounds(nstiles, GRP8)

            # preload data group 0 FIRST: the sync HWDGE ring is FIFO per
            # engine, so anything emitted before these delays the first
            # matmul by its transfer+receipt time
            b0, n = xg_bounds[0]
            cur_xt = xgp.tile([128, GRP, C], dtx, tag="xg")
            nc.sync.dma_start(cur_xt[:, :n, :], xg_d[:, b0:b0 + n, :])
            xg_map = {b0 + q: q for q in range(n)}
            xg_next = 1
            b0, n = gq_bounds[0]
            cur_gq = gqp.tile([128, GRP8, 128], dt8, tag="gq")
            nc.sync.dma_start(cur_gq[:, :n, :], gq_d[:, b0:b0 + n, :])
            gq_map = {b0 + q: q for q in range(n)}
            gq_next = 1

            iota_t = constp.tile([128, 128], dtb)
            nc.gpsimd.iota(iota_t[:], [[1, 128]], channel_multiplier=0,
                           allow_small_or_imprecise_dtypes=True)
            off_sb = constp.tile([128, ntp], dtf)
            nc.sync.dma_start(off_sb[:], off_d[:])
            w1_sb = constp.tile([C, C], dtb)
            nc.sync.dma_start(w1_sb[:], w1_d[:])
            w2c_sb = constp.tile([C, 1], dtb)
            nc.sync.dma_start(w2c_sb[:], w2c_d[:])

            hxT_sb = constp.tile([128, NFREE], dtb)
            hw0_sb = constp.tile([1, NFREE], dtf)

            def w1_strip(s0, w):
                # hT strip = W1^T @ hxT[:, s0:s0+w], relu, then
                # hw0 strip = w2col^T @ reluT (M=1 stationary, cheap ld)
                psh = ps_h.tile([C, 512], dtf, tag="h")
                nc.tensor.matmul(psh[:, :w], w1_sb[:], hxT_sb[:, s0:s0 + w],
                                 start=True, stop=True)
                reluT_sb = rlp.tile([128, 512], dtb, tag="reluT")
                nc.scalar.activation(reluT_sb[:, :w], psh[:, :w], AF.Relu)
                pso = ps_o.tile([1, 512], dtf, tag="o")
                nc.tensor.matmul(pso[:, :w], w2c_sb[:], reluT_sb[:, :w],
                                 start=True, stop=True)
                nc.scalar.activation(hw0_sb[:, s0:s0 + w], pso[:, :w],
                                     AF.Copy)

            t = 0
            s = 0
            for j in range(NSLOT):
                r = rj[j]
                psum_hx = ps_hx.tile([C, BLK], dtf, tag="hx")
                for i in range(r):
                    if xg_next < len(xg_bounds) and t == xg_bounds[xg_next][0]:
                        b0, n = xg_bounds[xg_next]
                        cur_xt = xgp.tile([128, GRP, C], dtx, tag="xg")
                        nc.sync.dma_start(cur_xt[:, :n, :],
                                          xg_d[:, b0:b0 + n, :])
                        xg_map = {b0 + q: q for q in range(n)}
                        xg_next += 1
                    if _shipped(t):
                        if gq_next < len(gq_bounds) and \
                                s == gq_bounds[gq_next][0]:
                            b0, n = gq_bounds[gq_next]
                            cur_gq = gqp.tile([128, GRP8, 128], dt8, tag="gq")
                            nc.sync.dma_start(cur_gq[:, :n, :],
                                              gq_d[:, b0:b0 + n, :])
                            gq_map = {b0 + q: q for q in range(n)}
                            gq_next += 1
                        g_ap = cur_gq[:, gq_map[s], :]
                        s += 1
                    else:
                        g_sb = gp.tile([128, 128], dtb, tag="G")
                        nc.vector.tensor_scalar(
                            g_sb[:], iota_t[:], off_sb[:, t:t + 1], None,
                            mybir.AluOpType.is_equal)
                        g_ap = g_sb[:]
                    # psum_hx[c, j] += sum_k xg[k, c] * G[k, j]
                    nc.tensor.matmul(psum_hx[:], cur_xt[:, xg_map[t], :],
                                     g_ap, start=(i == 0), stop=(i == r - 1))
                    t += 1
                nc.scalar.activation(hxT_sb[:, j * BLK:(j + 1) * BLK],
                                     psum_hx[:], AF.Copy)
                if (j + 1) % WSTRIP == 0:
                    w1_strip((j + 1 - WSTRIP) * BLK, WSTRIP * BLK)
            rem = NSLOT % WSTRIP
            if rem:
                w1_strip((NSLOT - rem) * BLK, rem * BLK)

            nc.sync.dma_start(hw0_d[:], hw0_sb[:])

    nc.compile()
    return nc


def _build_prog_b(free):
    """Layer-2 program (raw bass, minimal tail):
    acc[p] = sum_f sigmoid(0.5*(a+b)).  zab is [za | zb] along free."""
    import concourse.bass as bass
    import concourse.mybir as mybir

    dtb = mybir.dt.bfloat16
    dtf = mybir.dt.float32
    AF = mybir.ActivationFunctionType

    nc = bass.Bass()
    zab_d = nc.dram_tensor("zab", [128, 2 * free], dtb, kind="ExternalInput")
    acc_d = nc.dram_tensor("acc", [128, 1], dtf, kind="ExternalOutput")

    with (
        nc.sbuf_tensor([128, 2 * free], dtb) as zab_sb,
        nc.sbuf_tensor([128, free], dtf) as t_sb,
        nc.sbuf_tensor([128, free], dtf) as s_sb,
        nc.sbuf_tensor([128, 1], dtf) as r_sb,
        nc.semaphore() as dsem,
        nc.semaphore() as csem,
        nc.Block() as block,
    ):
        @block.sync
        def _(sync):
            sync.dma_start(zab_sb[:], zab_d[:]).then_inc(dsem, 16)
            sync.wait_ge(csem, 2)
            sync.dma_start(acc_d[:], r_sb[:]).then_inc(dsem, 16)

        @block.vector
        def _(vector):
            vector.wait_ge(dsem, 16)
            nc.vector.tensor_add(t_sb[:], zab_sb[:, :free],
                                 zab_sb[:, free:]).then_inc(csem, 1)

        @block.scalar
        def _(scalar):
            scalar.wait_ge(csem, 1)
            nc.scalar.activation(s_sb[:], t_sb[:], AF.Sigmoid, scale=0.5,
                                 accum_out=r_sb[:]).then_inc(csem, 1)

    return nc


def _get_prog(key, builder, *args):
    if key not in _PROGS:
        _PROGS[key] = builder(*args)
    return _PROGS[key]


def _run(nc, in_maps, tag):
    from concourse.bass_utils import run_bass_kernel_spmd
    import time

    t0 = time.perf_counter()
    res = run_bass_kernel_spmd(nc, in_maps, list(range(NCORES)), trace=TRACE)
    LAST[tag + "_wall_s"] = time.perf_counter() - t0
    LAST[tag + "_exec_ns"] = res.exec_time_ns
    return res.results


def kernel(x, w1, w2, inc_rows, inc_cols, n_nodes=None, n_edges=None):
    x = np.asarray(x, dtype=np.float32)
    w1 = np.asarray(w1, dtype=np.float32)
    w2 = np.asarray(w2, dtype=np.float32)
    inc_rows = np.asarray(inc_rows)
    inc_cols = np.asarray(inc_cols)
    assert x.shape == (N_EDGES, C) and inc_rows.shape == (NNZ,)
    # every edge contributes exactly its two adjacent nonzeros (deg == 2)
    assert np.array_equal(inc_cols.astype(np.int64),
                          np.arange(NNZ, dtype=np.int64) // 2)

    # ---- host prep for launch A: sort nnz by destination node ----
    order = np.argsort(inc_rows, kind="stable")
    rs = inc_rows[order].astype(np.int64)
    cs = inc_cols[order].astype(np.int64)

    blk = rs >> 7
    counts = np.bincount(blk, minlength=NBLK)
    starts = np.zeros(NBLK, np.int64)
    starts[1:] = np.cumsum(counts)[:-1]

    # sorted block -> (core, slot) assignment; per-slot tile count rj
    ordb = np.argsort(-counts, kind="stable")          # NBLK block ids
    pos = np.empty(NBLK, np.int64)
    pos[ordb] = np.arange(NBLK)                        # block -> rank
    slot_of_blk = pos // NCORES
    core_of_blk = pos % NCORES
    slot_counts = counts[ordb].reshape(NSLOT, NCORES)
    rj = np.maximum(1, -(-slot_counts.max(axis=1) // 128)).astype(int)
    toff = np.zeros(NSLOT, np.int64)
    toff[1:] = np.cumsum(rj)[:-1]
    T = int(rj.sum())
    NTP = -(-T // GRP) * GRP
    ship_mask_t = np.array([_shipped(t) for t in range(T)], dtype=bool)
    ship_idx_t = np.cumsum(ship_mask_t) - 1            # tile -> gq slot
    NSH = int(ship_mask_t.sum())
    NSHIP = -(-NSH // GRP8) * GRP8

    # per-nnz destination coordinates
    k = np.arange(NNZ, dtype=np.int64)
    w_in_blk = k - starts[blk]
    core_k = core_of_blk[blk]
    slot_k = slot_of_blk[blk]
    tile_k = toff[slot_k] + (w_in_blk >> 7)            # per-core tile index
    p_k = w_in_blk & 127
    o_k = (rs & 127)

    xdt = ml_dtypes.float8_e4m3 if XG_FP8 else ml_dtypes.bfloat16
    xbf = x.astype(xdt)
    xg_cores = np.zeros((NCORES, 128, NTP, C), dtype=xdt)
    xg_cores[core_k, p_k, tile_k, :] = xbf[cs]
    off_cores = np.zeros((NCORES, 128, NTP), dtype=np.float32)
    off_cores[core_k, p_k, tile_k] = o_k.astype(np.float32)

    gq_cores = np.zeros((NCORES, 128, NSHIP, 128), dtype=np.uint8)
    shipped_k = ship_mask_t[tile_k]
    gq_cores[core_k[shipped_k], p_k[shipped_k],
             ship_idx_t[tile_k[shipped_k]], o_k[shipped_k]] = FP8_ONE
    gq_cores = gq_cores.view(ml_dtypes.float8_e4m3)

    w1b = w1.astype(ml_dtypes.bfloat16)
    w2cb = w2[:, 0:1].astype(ml_dtypes.bfloat16)

    prog_a = _get_prog(("A", tuple(rj), NTP, NSHIP), _build_prog_a,
                       rj, NTP, NSHIP, T, NSH)
    in_maps = [{"xg": xg_cores[m], "off": off_cores[m], "gq": gq_cores[m],
                "w1": w1b, "w2c": w2cb} for m in range(NCORES)]
    res_a = _run(prog_a, in_maps, "A")

    # ---- host glue: assemble hw0, gather per-nonzero values ----
    # per-core hw0 row: [1, 6272], local node = 128*slot + p
    parts = np.stack([res_a[m]["hw0"].reshape(NSLOT, 128)
                      for m in range(NCORES)])                  # [8,49,128]
    by_rank = parts.transpose(1, 0, 2).reshape(NBLK, 128)       # rank-major
    hw0 = np.empty((NBLK, 128), dtype=np.float32)
    hw0[ordb] = by_rank
    hw0 = hw0.reshape(-1)
    zg = hw0[inc_rows.astype(np.int64)]
    za = zg[0::2]
    zb = zg[1::2]

    # ---- launch B: sigmoid + reduce ----
    FREE = -(-N_EDGES // (NCORES * 128))               # 196
    tot = NCORES * 128 * FREE
    zap = np.full(tot, -1.0e4, np.float32)
    zbp = np.full(tot, -1.0e4, np.float32)
    zap[:N_EDGES] = za
    zbp[:N_EDGES] = zb
    zab = np.concatenate(
        [zap.reshape(NCORES, 128, FREE), zbp.reshape(NCORES, 128, FREE)],
        axis=2).astype(ml_dtypes.bfloat16)

    prog_b = _get_prog(("B", FREE), _build_prog_b, FREE)
    in_maps_b = [{"zab": zab[m]} for m in range(NCORES)]
    res_b = _run(prog_b, in_maps_b, "B")

    total = float(sum(float(r["acc"].sum()) for r in res_b))
    return np.array(total / N_EDGES, dtype=np.float32)


# revision 47
# speedup vs baseline: 1.1979x; 1.0201x over previous
"""Trainium2 Bass kernel for nn_DHGNNLayer (gnn_message_passing).

Math (from the reference):
    h   = relu(B1 @ x @ W1)            # [n_nodes, 128], B1 = COO incidence
    out = mean_e sigmoid((hw0[r_{2e}] + hw0[r_{2e+1}]) / 2)   # scalar
    where hw0 = relu(h) @ W2[:, 0]     # only column 0 is ever needed

Key facts used:
  - inc_cols == arange(NNZ)//2  -> every edge has exactly 2 nonzeros, deg == 2,
    and the two nonzeros of edge e are adjacent (2e, 2e+1) in original order.

Strategy (8 cores, 1D node-partition parallelism, no collectives):
  Launch A: host sorts nonzeros by destination node and gathers x rows per
    nonzero (fp8e4m3).  Nodes are split into 128-wide blocks; blocks are sorted
    by nnz count and dealt round-robin to (core, slot) so every core runs an
    identical program (SPMD) with per-slot tile counts R_j.  Per 128-nnz
    tile, a one-hot G[k, j] = (off[k] == j) is either built on the DVE
    (iota + tensor_scalar is_equal) or shipped from the host as fp8 (exact
    0/1) — a tunable DMA-vs-DVE tradeoff.  The tensor engine accumulates
    hxT_block += xg_tile^T @ G in PSUM.  Then hT = W1^T @ hxT (stationary
    W1), ReLU, and hw0_block = reluT_block^T @ W2[:,0].
  Launch B: host gathers hw0[inc_rows] (1.6 MB), device does
    sigmoid(0.5*(a+b)) and reduces; host combines 8 partial sums.
"""

import numpy as np
import ml_dtypes

N_NODES = 50000
N_EDGES = 200000
C = 128
NNZ = 2 * N_EDGES
NCORES = 8
BLK = 128                      # nodes per block (PSUM window)
NBLK = 392                     # ceil(50000/128) padded to a multiple of 8
NSLOT = NBLK // NCORES         # 49 node blocks (slots) per core
NODES_PAD = NBLK * BLK         # 50176
GRP = 64                       # xg tiles per DMA group (1 MiB fp8)
GRP8 = 64                      # gq tiles per DMA group (1 MiB fp8)
SHIP_NUM = 3                   # ship G for tiles with t % SHIP_DEN < SHIP_NUM
SHIP_DEN = 5
XG_FP8 = True                  # ship xg as fp8e4m3 (halves the big DMA)
WSTRIP = 4                     # slots per w1/relu strip
FP8_ONE = np.uint8(0x38)       # float8_e4m3 encoding of 1.0

_PROGS = {}
TRACE = False
LAST = {}


def _shipped(t):
    return (t % SHIP_DEN) < SHIP_NUM


def _bacc():
    import concourse.bacc as bacc

    return bacc.Bacc("TRN2", target_bir_lowering=False, debug=False,
                     num_devices=NCORES)


def _build_prog_a(rj, ntp, nship, ntiles, nstiles):
    """Layer-1 program: segment-sum + W1 + relu + W2[:,0] per node block."""
    import concourse.mybir as mybir
    from concourse import tile

    dtb = mybir.dt.bfloat16
    dtf = mybir.dt.float32
    dt8 = mybir.dt.float8e4
    dtx = dt8 if XG_FP8 else dtb
    AF = mybir.ActivationFunctionType
    NFREE = NSLOT * BLK        # 6272 nodes per core

    nc = _bacc()
    xg_d = nc.dram_tensor("xg", [128, ntp, C], dtx, kind="ExternalInput")
    off_d = nc.dram_tensor("off", [128, ntp], dtf, kind="ExternalInput")
    gq_d = nc.dram_tensor("gq", [128, nship, 128], dt8, kind="ExternalInput")
    w1_d = nc.dram_tensor("w1", [C, C], dtb, kind="ExternalInput")
    w2c_d = nc.dram_tensor("w2c", [C, 1], dtb, kind="ExternalInput")
    hw0_d = nc.dram_tensor("hw0", [1, NFREE], dtf, kind="ExternalOutput")

    with tile.TileContext(nc) as tc:
        with (
            tc.tile_pool(name="const", bufs=1) as constp,
            tc.tile_pool(name="xgp", bufs=4) as xgp,
            tc.tile_pool(name="gqp", bufs=4) as gqp,
            tc.tile_pool(name="gp", bufs=16) as gp,
            tc.tile_pool(name="rlp", bufs=4) as rlp,
            tc.tile_pool(name="ps_hx", bufs=4, space="PSUM") as ps_hx,
            tc.tile_pool(name="ps_h", bufs=2, space="PSUM") as ps_h,
            tc.tile_pool(name="ps_o", bufs=2, space="PSUM") as ps_o,
        ):
            def bounds(total, full):
                out = []
                b = 0
                while b < total:
                    n = min(full, total - b)
                    out.append((b, n))
                    b += n
                return out

            xg_bounds = bounds(ntiles, GRP)
            gq_bounds = bounds(nstiles, GRP8)

            # DMA order matters: the sync HWDGE ring is FIFO per engine.
            # off first (small, unblocks the DVE G-builds — the critical
            # engine), then data group 0 for the first matmuls, then the
            # weights (not needed until the first strip).
            iota_t = constp.tile([128, 128], dtb)
            nc.gpsimd.iota(iota_t[:], [[1, 128]], channel_multiplier=0,
                           allow_small_or_imprecise_dtypes=True)
            off_sb = constp.tile([128, ntp], dtf)
            nc.sync.dma_start(off_sb[:], off_d[:])
            b0, n = xg_bounds[0]
            cur_xt = xgp.tile([128, GRP, C], dtx, tag="xg")
            nc.sync.dma_start(cur_xt[:, :n, :], xg_d[:, b0:b0 + n, :])
            xg_map = {b0 + q: q for q in range(n)}
            xg_next = 1
            b0, n = gq_bounds[0]
            cur_gq = gqp.tile([128, GRP8, 128], dt8, tag="gq")
            nc.sync.dma_start(cur_gq[:, :n, :], gq_d[:, b0:b0 + n, :])
            gq_map = {b0 + q: q for q in range(n)}
            gq_next = 1
            w1_sb = constp.tile([C, C], dtb)
            nc.sync.dma_start(w1_sb[:], w1_d[:])
            w2c_sb = constp.tile([C, 1], dtb)
            nc.sync.dma_start(w2c_sb[:], w2c_d[:])

            hxT_sb = constp.tile([128, NFREE], dtb)
            hw0_sb = constp.tile([1, NFREE], dtf)

            def w1_strip(s0, w):
                # hT strip = W1^T @ hxT[:, s0:s0+w], relu, then
                # hw0 strip = w2col^T @ reluT (M=1 stationary, cheap ld)
                psh = ps_h.tile([C, 512], dtf, tag="h")
                nc.tensor.matmul(psh[:, :w], w1_sb[:], hxT_sb[:, s0:s0 + w],
                                 start=True, stop=True)
                reluT_sb = rlp.tile([128, 512], dtb, tag="reluT")
                nc.scalar.activation(reluT_sb[:, :w], psh[:, :w], AF.Relu)
                pso = ps_o.tile([1, 512], dtf, tag="o")
                nc.tensor.matmul(pso[:, :w], w2c_sb[:], reluT_sb[:, :w],
                                 start=True, stop=True)
                nc.scalar.activation(hw0_sb[:, s0:s0 + w], pso[:, :w],
                                     AF.Copy)

            t = 0
            s = 0
            for j in range(NSLOT):
                r = rj[j]
                psum_hx = ps_hx.tile([C, BLK], dtf, tag="hx")
                for i in range(r):
                    if xg_next < len(xg_bounds) and t == xg_bounds[xg_next][0]:
                        b0, n = xg_bounds[xg_next]
                        cur_xt = xgp.tile([128, GRP, C], dtx, tag="xg")
                        nc.sync.dma_start(cur_xt[:, :n, :],
                                          xg_d[:, b0:b0 + n, :])
                        xg_map = {b0 + q: q for q in range(n)}
                        xg_next += 1
                    if _shipped(t):
                        if gq_next < len(gq_bounds) and \
                                s == gq_bounds[gq_next][0]:
                            b0, n = gq_bounds[gq_next]
                            cur_gq = gqp.tile([128, GRP8, 128], dt8, tag="gq")
                            nc.sync.dma_start(cur_gq[:, :n, :],
                                              gq_d[:, b0:b0 + n, :])
                            gq_map = {b0 + q: q for q in range(n)}
                            gq_next += 1
                        g_ap = cur_gq[:, gq_map[s], :]
                        s += 1
                    else:
                        g_sb = gp.tile([128, 128], dtb, tag="G")
                        nc.vector.tensor_scalar(
                            g_sb[:], iota_t[:], off_sb[:, t:t + 1], None,
                            mybir.AluOpType.is_equal)
                        g_ap = g_sb[:]
                    # psum_hx[c, j] += sum_k xg[k, c] * G[k, j]
                    nc.tensor.matmul(psum_hx[:], cur_xt[:, xg_map[t], :],
                                     g_ap, start=(i == 0), stop=(i == r - 1))
                    t += 1
                nc.scalar.activation(hxT_sb[:, j * BLK:(j + 1) * BLK],
                                     psum_hx[:], AF.Copy)
                if (j + 1) % WSTRIP == 0:
                    w1_strip((j + 1 - WSTRIP) * BLK, WSTRIP * BLK)
            rem = NSLOT % WSTRIP
            if rem:
                w1_strip((NSLOT - rem) * BLK, rem * BLK)

            nc.sync.dma_start(hw0_d[:], hw0_sb[:])

    nc.compile()
    return nc


def _build_prog_b(free):
    """Layer-2 program (raw bass, minimal tail):
    acc[p] = sum_f sigmoid(0.5*(a+b)).  zab is [za | zb] along free."""
    import concourse.bass as bass
    import concourse.mybir as mybir

    dtb = mybir.dt.bfloat16
    dtf = mybir.dt.float32
    AF = mybir.ActivationFunctionType

    nc = bass.Bass()
    zab_d = nc.dram_tensor("zab", [128, 2 * free], dtb, kind="ExternalInput")
    acc_d = nc.dram_tensor("acc", [128, 1], dtf, kind="ExternalOutput")

    with (
        nc.sbuf_tensor([128, 2 * free], dtb) as zab_sb,
        nc.sbuf_tensor([128, free], dtf) as t_sb,
        nc.sbuf_tensor([128, free], dtf) as s_sb,
        nc.sbuf_tensor([128, 1], dtf) as r_sb,
        nc.semaphore() as dsem,
        nc.semaphore() as csem,
        nc.Block() as block,
    ):
        @block.sync
        def _(sync):
            sync.dma_start(zab_sb[:], zab_d[:]).then_inc(dsem, 16)
            sync.wait_ge(csem, 2)
            sync.dma_start(acc_d[:], r_sb[:]).then_inc(dsem, 16)

        @block.vector
        def _(vector):
            vector.wait_ge(dsem, 16)
            nc.vector.tensor_add(t_sb[:], zab_sb[:, :free],
                                 zab_sb[:, free:]).then_inc(csem, 1)

        @block.scalar
        def _(scalar):
            scalar.wait_ge(csem, 1)
            nc.scalar.activation(s_sb[:], t_sb[:], AF.Sigmoid, scale=0.5,
                                 accum_out=r_sb[:]).then_inc(csem, 1)

    return nc


def _get_prog(key, builder, *args):
    if key not in _PROGS:
        _PROGS[key] = builder(*args)
    return _PROGS[key]


def _run(nc, in_maps, tag):
    from concourse.bass_utils import run_bass_kernel_spmd
    import time

    t0 = time.perf_counter()
    res = run_bass_kernel_spmd(nc, in_maps, list(range(NCORES)), trace=TRACE)
    LAST[tag + "_wall_s"] = time.perf_counter() - t0
    LAST[tag + "_exec_ns"] = res.exec_time_ns
    return res.results


def kernel(x, w1, w2, inc_rows, inc_cols, n_nodes=None, n_edges=None):
    x = np.asarray(x, dtype=np.float32)
    w1 = np.asarray(w1, dtype=np.float32)
    w2 = np.asarray(w2, dtype=np.float32)
    inc_rows = np.asarray(inc_rows)
    inc_cols = np.asarray(inc_cols)
    assert x.shape == (N_EDGES, C) and inc_rows.shape == (NNZ,)
    # every edge contributes exactly its two adjacent nonzeros (deg == 2)
    assert np.array_equal(inc_cols.astype(np.int64),
                          np.arange(NNZ, dtype=np.int64) // 2)

    # ---- host prep for launch A: sort nnz by destination node ----
    order = np.argsort(inc_rows, kind="stable")
    rs = inc_rows[order].astype(np.int64)
    cs = inc_cols[order].astype(np.int64)

    blk = rs >> 7
    counts = np.bincount(blk, minlength=NBLK)
    starts = np.zeros(NBLK, np.int64)
    starts[1:] = np.cumsum(counts)[:-1]

    # sorted block -> (core, slot) assignment; per-slot tile count rj
    ordb = np.argsort(-counts, kind="stable")          # NBLK block ids
    pos = np.empty(NBLK, np.int64)
    pos[ordb] = np.arange(NBLK)                        # block -> rank
    slot_of_blk = pos // NCORES
    core_of_blk = pos % NCORES
    slot_counts = counts[ordb].reshape(NSLOT, NCORES)
    rj = np.maximum(1, -(-slot_counts.max(axis=1) // 128)).astype(int)
    toff = np.zeros(NSLOT, np.int64)
    toff[1:] = np.cumsum(rj)[:-1]
    T = int(rj.sum())
    NTP = -(-T // GRP) * GRP
    ship_mask_t = np.array([_shipped(t) for t in range(T)], dtype=bool)
    ship_idx_t = np.cumsum(ship_mask_t) - 1            # tile -> gq slot
    NSH = int(ship_mask_t.sum())
    NSHIP = -(-NSH // GRP8) * GRP8

    # per-nnz destination coordinates
    k = np.arange(NNZ, dtype=np.int64)
    w_in_blk = k - starts[blk]
    core_k = core_of_blk[blk]
    slot_k = slot_of_blk[blk]
    tile_k = toff[slot_k] + (w_in_blk >> 7)            # per-core tile index
    p_k = w_in_blk & 127
    o_k = (rs & 127)

    xdt = ml_dtypes.float8_e4m3 if XG_FP8 else ml_dtypes.bfloat16
    xbf = x.astype(xdt)
    xg_cores = np.zeros((NCORES, 128, NTP, C), dtype=xdt)
    xg_cores[core_k, p_k, tile_k, :] = xbf[cs]
    off_cores = np.zeros((NCORES, 128, NTP), dtype=np.float32)
    off_cores[core_k, p_k, tile_k] = o_k.astype(np.float32)

    gq_cores = np.zeros((NCORES, 128, NSHIP, 128), dtype=np.uint8)
    shipped_k = ship_mask_t[tile_k]
    gq_cores[core_k[shipped_k], p_k[shipped_k],
             ship_idx_t[tile_k[shipped_k]], o_k[shipped_k]] = FP8_ONE
    gq_cores = gq_cores.view(ml_dtypes.float8_e4m3)

    w1b = w1.astype(ml_dtypes.bfloat16)
    w2cb = w2[:, 0:1].astype(ml_dtypes.bfloat16)

    prog_a = _get_prog(("A", tuple(rj), NTP, NSHIP), _build_prog_a,
                       rj, NTP, NSHIP, T, NSH)
    in_maps = [{"xg": xg_cores[m], "off": off_cores[m], "gq": gq_cores[m],
                "w1": w1b, "w2c": w2cb} for m in range(NCORES)]
    res_a = _run(prog_a, in_maps, "A")

    # ---- host glue: assemble hw0, gather per-nonzero values ----
    # per-core hw0 row: [1, 6272], local node = 128*slot + p
    parts = np.stack([res_a[m]["hw0"].reshape(NSLOT, 128)
                      for m in range(NCORES)])                  # [8,49,128]
    by_rank = parts.transpose(1, 0, 2).reshape(NBLK, 128)       # rank-major
    hw0 = np.empty((NBLK, 128), dtype=np.float32)
    hw0[ordb] = by_rank
    hw0 = hw0.reshape(-1)
    zg = hw0[inc_rows.astype(np.int64)]
    za = zg[0::2]
    zb = zg[1::2]

    # ---- launch B: sigmoid + reduce ----
    FREE = -(-N_EDGES // (NCORES * 128))               # 196
    tot = NCORES * 128 * FREE
    zap = np.full(tot, -1.0e4, np.float32)
    zbp = np.full(tot, -1.0e4, np.float32)
    zap[:N_EDGES] = za
    zbp[:N_EDGES] = zb
    zab = np.concatenate(
        [zap.reshape(NCORES, 128, FREE), zbp.reshape(NCORES, 128, FREE)],
        axis=2).astype(ml_dtypes.bfloat16)

    prog_b = _get_prog(("B", FREE), _build_prog_b, FREE)
    in_maps_b = [{"zab": zab[m]} for m in range(NCORES)]
    res_b = _run(prog_b, in_maps_b, "B")

    total = float(sum(float(r["acc"].sum()) for r in res_b))
    return np.array(total / N_EDGES, dtype=np.float32)
